# revision 20
# baseline (speedup 1.0000x reference)
"""Trainium2 Bass kernel for ComputeAllAtomCoords (nn_ComputeAllAtomCoords).

Full inputs in, full outputs out. Internally: data-parallel over residues L
across 8 NeuronCores (sharding hint). Per core:
  - residues laid out SoA: residue = (partition p, free col f), l = p*F + f
  - torsion rotX normalization on DVE+ACT
  - per-residue RT matrices gathered from the 5-entry table via one-hot
    PE matmul (+PE transpose back to SoA layout)
  - frame chain RTF_k = parent @ RT_k @ rotX_k on DVE with broadcast-AP
    fused tensor_tensor ops
  - atom coords via 5 seq-masked PE matmuls with constant weights
    (xb points and base-frame indices baked into the weights host-side)
"""

import sys

sys.path.insert(0, "/opt/trn_rl_repo")

import numpy as np

import concourse.bass as bass
import concourse.bacc as bacc
import concourse.mybir as mybir
from concourse.bass_utils import run_bass_kernel_spmd
from concourse.tile import TileContext

F32 = mybir.dt.float32
OP = mybir.AluOpType

P = 128          # partitions
F = 49           # free columns per tile
NT = 4           # tiles per core
PF = P * F       # residues per tile (6272)
LCORE_PAD = PF * NT   # 25088
L = 200000
NC = 8
LCORE = L // NC  # 25000
EPS = 1e-6

# chain: frames 1..8 = step(parent_state, torsion k=sidx+2)
PARENT = [0, 1, 2, 3, 3, 5, 6, 6]

_CACHE = {}


def _build_bass():
    nc = bacc.Bacc()

    # DRAM I/O (per-core shapes)
    alph = nc.dram_tensor("alph", [LCORE_PAD, 16], F32, kind="ExternalInput")
    rst = nc.dram_tensor("rst", [LCORE_PAD, 12], F32, kind="ExternalInput")
    seqp = nc.dram_tensor("seqp", [NT, 5, PF], F32, kind="ExternalInput")
    rtt = nc.dram_tensor("rtt", [5, 128], F32, kind="ExternalInput")
    wat = nc.dram_tensor("wat", [108, 510], F32, kind="ExternalInput")
    mskw = nc.dram_tensor("mskw", [5, 540], F32, kind="ExternalInput")
    iden = nc.dram_tensor("iden", [128, 128], F32, kind="ExternalInput")
    iot5 = nc.dram_tensor("iot5", [5, 1], F32, kind="ExternalInput")
    frames = nc.dram_tensor("frames", [LCORE_PAD, 144], F32, kind="ExternalOutput")
    xyzo = nc.dram_tensor("xyzo", [LCORE_PAD, 102], F32, kind="ExternalOutput")

    with TileContext(nc) as tc:
        with (
            tc.tile_pool(name="const", bufs=1) as cpool,
            tc.tile_pool(name="big", bufs=1) as bpool,
            tc.tile_pool(name="ld", bufs=2) as lpool,
            tc.tile_pool(name="small", bufs=2) as spool,
            tc.tile_pool(name="blk", bufs=2) as kpool,
            tc.tile_pool(name="ft", bufs=2) as fpool,
            tc.tile_pool(name="psum", bufs=1, space="PSUM") as ppool,
            tc.tile_pool(name="psum2", bufs=2, space="PSUM") as ppool2,
        )            :
            # --- constants into SBUF ---
            rtt_sb = cpool.tile([5, 128], F32, tag="rtt")
            nc.sync.dma_start(rtt_sb[:], rtt[:, :])
            wat_sb = cpool.tile([108, 510], F32, tag="wat")
            nc.sync.dma_start(wat_sb[:], wat[:, :])
            mskw_sb = cpool.tile([5, 540], F32, tag="mskw")
            nc.sync.dma_start(mskw_sb[:], mskw[:, :])
            iden_sb = cpool.tile([128, 128], F32, tag="iden")
            nc.sync.dma_start(iden_sb[:], iden[:, :])
            iot5_sb = cpool.tile([5, 1], F32, tag="iot5")
            nc.sync.dma_start(iot5_sb[:], iot5[:, :])

            for t in range(NT):
                r0 = t * PF
                # ---- loads ----
                ang = lpool.tile([P, F, 16], F32, tag="ang")
                nc.sync.dma_start(
                    ang[:],
                    alph[r0 : r0 + PF, :].rearrange("(p f) c -> p f c", f=F),
                )
                # states: (P, F, 9, 16) — frame k comps contiguous per residue
                states = bpool.tile([P, F, 9, 16], F32, tag="states")
                rstg = lpool.tile([P, F, 12], F32, tag="rstg")
                nc.sync.dma_start(
                    rstg[:], rst[r0 : r0 + PF, :].rearrange("(p f) c -> p f c", f=F)
                )
                st0r = states[:, :, 0, :].rearrange("p f (i m) -> p f i m", m=4)
                rr = rstg[:, :, 0:9].rearrange("p f (i j) -> p f i j", j=3)
                nc.vector.tensor_copy(st0r[:, :, 0:3, 0:3], rr)
                nc.vector.tensor_copy(st0r[:, :, 0:3, 3], rstg[:, :, 9:12])
                nc.gpsimd.memset(states[:, :, 0, 12:15], 0.0)
                nc.gpsimd.memset(states[:, :, 0, 15:16], 1.0)

                oht = bpool.tile([5, PF], F32, tag="oht")
                nc.sync.dma_start(oht[:], seqp[t])

                # ---- rotX normalization ----
                c_ap = ang[:, :, 0:16:2]
                s_ap = ang[:, :, 1:16:2]
                sq = spool.tile([P, F, 8], F32, tag="sq")
                n2 = spool.tile([P, F, 8], F32, tag="n2")
                rec = spool.tile([P, F, 8], F32, tag="rec")
                nc.vector.tensor_tensor(sq[:], s_ap, s_ap, OP.mult)
                nc.vector.tensor_tensor(n2[:], c_ap, c_ap, OP.mult)
                nc.vector.tensor_tensor(n2[:], n2[:], sq[:], OP.add)
                nc.scalar.sqrt(sq[:], n2[:])
                nc.vector.tensor_scalar_add(sq[:], sq[:], EPS)
                nc.vector.reciprocal(rec[:], sq[:])
                angn = bpool.tile([P, F, 16], F32, tag="angn")
                nc.vector.tensor_tensor(angn[:, :, 0:16:2], c_ap, rec[:], OP.mult)
                nc.vector.tensor_tensor(angn[:, :, 1:16:2], s_ap, rec[:], OP.mult)

                # ---- one-hot over seq (5, PF), in place ----
                nc.vector.tensor_scalar(
                    oht[:], oht[:], iot5_sb[:, 0:1], None, OP.is_equal
                )

                # ---- RTbar gather: (5->128) matmul + transpose to SoA ----
                rtbar = bpool.tile([P, F, 8, 16], F32, tag="rtbar")
                nblk = (F + 3) // 4
                for b in range(nblk):
                    nf = min(4, F - b * 4)
                    nb = nf * 128
                    rb = ppool.tile([128, 512], F32, tag="rb")
                    nc.tensor.matmul(
                        rb[:, :nb],
                        rtt_sb[:],
                        oht[:, b * 512 : b * 512 + nb],
                        start=True,
                        stop=True,
                    )
                    rbs = kpool.tile([128, 512], F32, tag="rbs")
                    nc.scalar.copy(rbs[:, :nb], rb[:, :nb])
                    for fi in range(nf):
                        f_abs = b * 4 + fi
                        tp = ppool.tile([128, 128], F32, tag="tp")
                        nc.tensor.transpose(
                            tp[:], rbs[:, fi * 128 : (fi + 1) * 128], iden_sb[:]
                        )
                        nc.scalar.copy(
                            rtbar[:, f_abs],
                            tp[:].rearrange("p (k c) -> p k c", c=16),
                        )

                # ---- chain ----
                # feat: contiguous rows 0-2 of every frame, for the PE bridge
                feat = fpool.tile([P, F, 108], F32, tag="feat")
                ftr = feat[:, :, 0:12].rearrange("p f (i j) -> p f i j", j=4)
                nc.scalar.copy(ftr[:, :, :, 0:3], rr)
                nc.scalar.copy(ftr[:, :, :, 3], rstg[:, :, 9:12])
                for sidx in range(8):
                    Pst = states[:, :, PARENT[sidx]].rearrange(
                        "p f (i m) -> p f i m", m=4
                    )
                    C4 = states[:, :, sidx + 1].rearrange("p f (i m) -> p f i m", m=4)
                    B4 = rtbar[:, :, sidx, :].rearrange("p f (j m) -> p f j m", m=4)
                    tmp4a = spool.tile([P, F, 4, 4], F32, tag="tmp4a")
                    tmp4b = spool.tile([P, F, 4, 4], F32, tag="tmp4b")
                    for j in range(4):
                        pjb = Pst[:, :, :, j].unsqueeze(3).broadcast_to([P, F, 4, 4])
                        bjb = B4[:, :, j, :].unsqueeze(2).broadcast_to([P, F, 4, 4])
                        if j == 0:
                            nc.vector.tensor_tensor(C4[:], pjb, bjb, OP.mult)
                        else:
                            tmp4 = tmp4a if j % 2 else tmp4b
                            nc.vector.tensor_tensor(tmp4[:], pjb, bjb, OP.mult)
                            nc.gpsimd.tensor_tensor(C4[:], C4[:], tmp4[:], OP.add)
                    # right-multiply by rotX(sidx): cols 1,2 mix
                    cb = angn[:, :, 2 * sidx].unsqueeze(2).broadcast_to([P, F, 4])
                    sb = angn[:, :, 2 * sidx + 1].unsqueeze(2).broadcast_to([P, F, 4])
                    d1 = C4[:, :, :, 1]
                    d2 = C4[:, :, :, 2]
                    ta = spool.tile([P, F, 4], F32, tag="ta")
                    tb2 = spool.tile([P, F, 4], F32, tag="tb2")
                    tcc = spool.tile([P, F, 4], F32, tag="tcc")
                    td = spool.tile([P, F, 4], F32, tag="td")
                    nc.vector.tensor_tensor(ta[:], d1, cb, OP.mult)
                    nc.vector.tensor_tensor(tb2[:], d2, sb, OP.mult)
                    nc.vector.tensor_tensor(tcc[:], d2, cb, OP.mult)
                    nc.vector.tensor_tensor(td[:], d1, sb, OP.mult)
                    nc.vector.tensor_tensor(d1, ta[:], tb2[:], OP.add)
                    nc.vector.tensor_tensor(d2, tcc[:], td[:], OP.subtract)
                    k = sidx + 1
                    nc.scalar.copy(
                        feat[:, :, k * 12 : (k + 1) * 12],
                        states[:, :, k, 0:12],
                    )

                # ---- frames out ----
                nc.sync.dma_start(
                    frames[r0 : r0 + PF, :].rearrange("(p f) c -> p f c", f=F),
                    states[:].rearrange("p f k c -> p f (k c)"),
                )

                # ---- atoms ----
                xyzout = bpool.tile([P, F, 102], F32, tag="xyzout")
                for b in range(nblk):
                    nf = min(4, F - b * 4)
                    nb = nf * 128
                    tb = ppool.tile([108, 512], F32, tag="tb")
                    for fi in range(nf):
                        f_abs = b * 4 + fi
                        nc.tensor.transpose(
                            tb[:, fi * 128 : (fi + 1) * 128],
                            feat[:, f_abs, :],
                            iden_sb[:],
                        )
                    tsb = kpool.tile([108, 512], F32, tag="tsb")
                    nc.scalar.copy(tsb[:, :nb], tb[:, :nb])
                    xy = ppool.tile([102, 512], F32, tag="xy")
                    for s in range(5):
                        ms = ppool2.tile([108, 512], F32, tag="ms")
                        nc.tensor.matmul(
                            ms[:, :nb],
                            mskw_sb[:, s * 108 : (s + 1) * 108],
                            oht[:, b * 512 : b * 512 + nb],
                            start=True,
                            stop=True,
                        )
                        ph = kpool.tile([108, 512], F32, tag="ph")
                        nc.vector.tensor_tensor(
                            ph[:, :nb], tsb[:, :nb], ms[:, :nb], OP.mult
                        )
                        nc.tensor.matmul(
                            xy[:, :nb],
                            wat_sb[:, s * 102 : (s + 1) * 102],
                            ph[:, :nb],
                            start=(s == 0),
                            stop=(s == 4),
                        )
                    xys = kpool.tile([102, 512], F32, tag="xys")
                    nc.scalar.copy(xys[:, :nb], xy[:, :nb])
                    for fi in range(nf):
                        f_abs = b * 4 + fi
                        xt = ppool.tile([128, 102], F32, tag="xt")
                        nc.tensor.transpose(
                            xt[:],
                            xys[:, fi * 128 : (fi + 1) * 128],
                            iden_sb[0:102, 0:102],
                        )
                        nc.scalar.copy(xyzout[:, f_abs, :], xt[:])

                nc.sync.dma_start(
                    xyzo[r0 : r0 + PF, :].rearrange("(p f) c -> p f c", f=F),
                    xyzout[:],
                )

    nc.finalize()
    return nc


def _host_consts(xyzs_in_base_frame, RTs_in_base_frame, base_indices):
    xb = np.asarray(xyzs_in_base_frame, dtype=np.float32)      # (5,34,4)
    rts = np.asarray(RTs_in_base_frame, dtype=np.float32)      # (5,10,4,4)
    bi = np.asarray(base_indices).astype(np.int64)             # (5,34)

    rtt = np.zeros((5, 128), np.float32)
    for k2 in range(8):
        rtt[:, k2 * 16 : (k2 + 1) * 16] = rts[:, k2 + 2].reshape(5, 16)

    wat = np.zeros((108, 510), np.float32)
    for s in range(5):
        for a in range(34):
            k = int(bi[s, a])
            for i in range(3):
                for j in range(4):
                    wat[k * 12 + i * 4 + j, s * 102 + a * 3 + i] = xb[s, a, j]

    mskw = np.zeros((5, 540), np.float32)
    for s in range(5):
        mskw[s, s * 108 : (s + 1) * 108] = 1.0

    iden = np.eye(128, dtype=np.float32)
    iot5 = np.arange(5, dtype=np.float32).reshape(5, 1)
    return rtt, wat, mskw, iden, iot5


def prepare_in_maps(alphas, listRs, listTs, xyzs_in_base_frame,
                    RTs_in_base_frame, seq, base_indices):
    al = np.asarray(alphas, dtype=np.float32).reshape(L, 10, 2)[:, 2:10, :]
    al = np.ascontiguousarray(al).reshape(L, 16)
    rs = np.asarray(listRs, dtype=np.float32).reshape(L, 9)
    ts = np.asarray(listTs, dtype=np.float32).reshape(L, 3)
    rst = np.concatenate([rs, ts], axis=1)                     # (L,12)
    sq = np.asarray(seq).astype(np.float32)                    # (L,)

    rtt, wat, mskw, iden, iot5 = _host_consts(
        xyzs_in_base_frame, RTs_in_base_frame, base_indices
    )

    in_maps = []
    for c in range(NC):
        lo = c * LCORE
        a_pad = np.zeros((LCORE_PAD, 16), np.float32)
        a_pad[:LCORE] = al[lo : lo + LCORE]
        r_pad = np.zeros((LCORE_PAD, 12), np.float32)
        r_pad[:LCORE] = rst[lo : lo + LCORE]
        s_pad = np.zeros((LCORE_PAD,), np.float32)
        s_pad[:LCORE] = sq[lo : lo + LCORE]
        # seqp[t, :, f*128 + p] = seq[t*PF + p*F + f], replicated on 5 rows
        sq1 = s_pad.reshape(NT, P, F).transpose(0, 2, 1).reshape(NT, 1, PF)
        seqp = np.broadcast_to(sq1, (NT, 5, PF)).copy()
        in_maps.append(
            dict(alph=a_pad, rst=r_pad, seqp=seqp, rtt=rtt, wat=wat,
                 mskw=mskw, iden=iden, iot5=iot5)
        )
    return in_maps


def run(in_maps, trace=False):
    if "nc" not in _CACHE:
        _CACHE["nc"] = _build_bass()
    nc = _CACHE["nc"]
    return run_bass_kernel_spmd(nc, in_maps, core_ids=list(range(NC)), trace=trace)


def kernel(alphas, listRs, listTs, xyzs_in_base_frame, RTs_in_base_frame,
           seq, base_indices):
    in_maps = prepare_in_maps(
        alphas, listRs, listTs, xyzs_in_base_frame, RTs_in_base_frame,
        seq, base_indices,
    )
    res = run(in_maps).results

    fr = np.concatenate(
        [r["frames"][:LCORE].reshape(LCORE, 9, 4, 4) for r in res], axis=0
    )
    xy = np.concatenate(
        [r["xyzo"][:LCORE].reshape(LCORE, 34, 3) for r in res], axis=0
    )
    return fr[None], xy[None]


# revision 21
# speedup vs baseline: 1.3979x; 1.3979x over previous
"""Trainium2 Bass kernel for ComputeAllAtomCoords (nn_ComputeAllAtomCoords).

Full inputs in, full outputs out. Internally: data-parallel over residues L
across 8 NeuronCores (sharding hint). Per core:
  - residues laid out SoA: residue = (partition p, free col f), l = p*F + f
  - torsion rotX normalization on DVE+ACT
  - per-residue RT matrices gathered from the 5-entry table via one-hot
    PE matmul (+PE transpose back to SoA layout)
  - frame chain RTF_k = parent @ RT_k @ rotX_k on DVE with broadcast-AP
    fused tensor_tensor ops
  - atom coords via 5 seq-masked PE matmuls with constant weights
    (xb points and base-frame indices baked into the weights host-side)
"""

import sys

sys.path.insert(0, "/opt/trn_rl_repo")

import numpy as np

import concourse.bass as bass
import concourse.bacc as bacc
import concourse.mybir as mybir
from concourse.bass_utils import run_bass_kernel_spmd
from concourse.tile import TileContext

F32 = mybir.dt.float32
OP = mybir.AluOpType

P = 128          # partitions
F = 49           # free columns per tile
NT = 4           # tiles per core
PF = P * F       # residues per tile (6272)
LCORE_PAD = PF * NT   # 25088
L = 200000
NC = 8
LCORE = L // NC  # 25000
EPS = 1e-6

# chain: frames 1..8 = step(parent_state, torsion k=sidx+2)
PARENT = [0, 1, 2, 3, 3, 5, 6, 6]

_CACHE = {}


def _build_bass():
    nc = bacc.Bacc()

    # DRAM I/O (per-core shapes)
    alph = nc.dram_tensor("alph", [LCORE_PAD, 16], F32, kind="ExternalInput")
    rst = nc.dram_tensor("rst", [LCORE_PAD, 12], F32, kind="ExternalInput")
    seqp = nc.dram_tensor("seqp", [NT, 5, PF], F32, kind="ExternalInput")
    rtt = nc.dram_tensor("rtt", [5, 128], F32, kind="ExternalInput")
    wat = nc.dram_tensor("wat", [108, 510], F32, kind="ExternalInput")
    mskw = nc.dram_tensor("mskw", [5, 540], F32, kind="ExternalInput")
    iden = nc.dram_tensor("iden", [128, 128], F32, kind="ExternalInput")
    iot5 = nc.dram_tensor("iot5", [5, 1], F32, kind="ExternalInput")
    frames = nc.dram_tensor("frames", [LCORE_PAD, 144], F32, kind="ExternalOutput")
    xyzo = nc.dram_tensor("xyzo", [LCORE_PAD, 102], F32, kind="ExternalOutput")

    with TileContext(nc) as tc:
        with (
            tc.tile_pool(name="const", bufs=1) as cpool,
            tc.tile_pool(name="big", bufs=1) as bpool,
            tc.tile_pool(name="ld", bufs=2) as lpool,
            tc.tile_pool(name="small", bufs=2) as spool,
            tc.tile_pool(name="blk", bufs=2) as kpool,
            tc.tile_pool(name="ft", bufs=2) as fpool,
            tc.tile_pool(name="psum", bufs=1, space="PSUM") as ppool,
            tc.tile_pool(name="psum2", bufs=2, space="PSUM") as ppool2,
        )            :
            # --- constants into SBUF ---
            rtt_sb = cpool.tile([5, 128], F32, tag="rtt")
            nc.sync.dma_start(rtt_sb[:], rtt[:, :])
            wat_sb = cpool.tile([108, 510], F32, tag="wat")
            nc.sync.dma_start(wat_sb[:], wat[:, :])
            mskw_sb = cpool.tile([5, 540], F32, tag="mskw")
            nc.sync.dma_start(mskw_sb[:], mskw[:, :])
            iden_sb = cpool.tile([128, 128], F32, tag="iden")
            nc.sync.dma_start(iden_sb[:], iden[:, :])
            iot5_sb = cpool.tile([5, 1], F32, tag="iot5")
            nc.sync.dma_start(iot5_sb[:], iot5[:, :])

            for t in range(NT):
                r0 = t * PF
                # ---- loads ----
                ang = lpool.tile([P, F, 16], F32, tag="ang")
                nc.sync.dma_start(
                    ang[:],
                    alph[r0 : r0 + PF, :].rearrange("(p f) c -> p f c", f=F),
                )
                # states: (P, F, 9, 16) — frame k comps contiguous per residue
                states = bpool.tile([P, F, 9, 16], F32, tag="states")
                rstg = lpool.tile([P, F, 12], F32, tag="rstg")
                nc.sync.dma_start(
                    rstg[:], rst[r0 : r0 + PF, :].rearrange("(p f) c -> p f c", f=F)
                )
                st0r = states[:, :, 0, :].rearrange("p f (i m) -> p f i m", m=4)
                rr = rstg[:, :, 0:9].rearrange("p f (i j) -> p f i j", j=3)
                nc.vector.tensor_copy(st0r[:, :, 0:3, 0:3], rr)
                nc.vector.tensor_copy(st0r[:, :, 0:3, 3], rstg[:, :, 9:12])
                nc.gpsimd.memset(states[:, :, 0, 12:15], 0.0)
                nc.gpsimd.memset(states[:, :, 0, 15:16], 1.0)

                oht = bpool.tile([5, PF], F32, tag="oht")
                nc.sync.dma_start(oht[:], seqp[t])

                # ---- rotX normalization ----
                c_ap = ang[:, :, 0:16:2]
                s_ap = ang[:, :, 1:16:2]
                sq = spool.tile([P, F, 8], F32, tag="sq")
                n2 = spool.tile([P, F, 8], F32, tag="n2")
                rec = spool.tile([P, F, 8], F32, tag="rec")
                nc.vector.tensor_tensor(sq[:], s_ap, s_ap, OP.mult)
                nc.vector.tensor_tensor(n2[:], c_ap, c_ap, OP.mult)
                nc.vector.tensor_tensor(n2[:], n2[:], sq[:], OP.add)
                nc.scalar.sqrt(sq[:], n2[:])
                nc.vector.tensor_scalar_add(sq[:], sq[:], EPS)
                nc.vector.reciprocal(rec[:], sq[:])
                angn = bpool.tile([P, F, 16], F32, tag="angn")
                nc.vector.tensor_tensor(angn[:, :, 0:16:2], c_ap, rec[:], OP.mult)
                nc.vector.tensor_tensor(angn[:, :, 1:16:2], s_ap, rec[:], OP.mult)

                # ---- one-hot over seq (5, PF), in place ----
                nc.vector.tensor_scalar(
                    oht[:], oht[:], iot5_sb[:, 0:1], None, OP.is_equal
                )

                # ---- RTbar gather: (5->128) matmul + transpose to SoA ----
                rtbar = bpool.tile([P, F, 8, 16], F32, tag="rtbar")
                nblk = (F + 3) // 4
                for b in range(nblk):
                    nf = min(4, F - b * 4)
                    nb = nf * 128
                    rb = ppool.tile([128, 512], F32, tag="rb")
                    nc.tensor.matmul(
                        rb[:, :nb],
                        rtt_sb[:],
                        oht[:, b * 512 : b * 512 + nb],
                        start=True,
                        stop=True,
                    )
                    rbs = kpool.tile([128, 512], F32, tag="rbs")
                    nc.scalar.copy(rbs[:, :nb], rb[:, :nb])
                    for fi in range(nf):
                        f_abs = b * 4 + fi
                        tp = ppool.tile([128, 128], F32, tag="tp")
                        nc.tensor.transpose(
                            tp[:], rbs[:, fi * 128 : (fi + 1) * 128], iden_sb[:]
                        )
                        nc.scalar.copy(
                            rtbar[:, f_abs],
                            tp[:].rearrange("p (k c) -> p k c", c=16),
                        )

                # ---- chain ----
                # feat: contiguous rows 0-2 of every frame, for the PE bridge
                feat = fpool.tile([P, F, 108], F32, tag="feat")
                ftr = feat[:, :, 0:12].rearrange("p f (i j) -> p f i j", j=4)
                nc.scalar.copy(ftr[:, :, :, 0:3], rr)
                nc.scalar.copy(ftr[:, :, :, 3], rstg[:, :, 9:12])
                for sidx in range(8):
                    Pst = states[:, :, PARENT[sidx]].rearrange(
                        "p f (i m) -> p f i m", m=4
                    )
                    C4 = states[:, :, sidx + 1].rearrange("p f (i m) -> p f i m", m=4)
                    B4 = rtbar[:, :, sidx, :].rearrange("p f (j m) -> p f j m", m=4)
                    tmp4a = spool.tile([P, F, 4, 4], F32, tag="tmp4a")
                    tmp4b = spool.tile([P, F, 4, 4], F32, tag="tmp4b")
                    for j in range(4):
                        pjb = Pst[:, :, :, j].unsqueeze(3).broadcast_to([P, F, 4, 4])
                        bjb = B4[:, :, j, :].unsqueeze(2).broadcast_to([P, F, 4, 4])
                        if j == 0:
                            nc.vector.tensor_tensor(C4[:], pjb, bjb, OP.mult)
                        else:
                            tmp4 = tmp4a if j % 2 else tmp4b
                            nc.vector.tensor_tensor(tmp4[:], pjb, bjb, OP.mult)
                            nc.gpsimd.tensor_tensor(C4[:], C4[:], tmp4[:], OP.add)
                    # right-multiply by rotX(sidx): cols 1,2 mix
                    cb = angn[:, :, 2 * sidx].unsqueeze(2).broadcast_to([P, F, 4])
                    sb = angn[:, :, 2 * sidx + 1].unsqueeze(2).broadcast_to([P, F, 4])
                    d1 = C4[:, :, :, 1]
                    d2 = C4[:, :, :, 2]
                    ta = spool.tile([P, F, 4], F32, tag="ta")
                    tb2 = spool.tile([P, F, 4], F32, tag="tb2")
                    tcc = spool.tile([P, F, 4], F32, tag="tcc")
                    td = spool.tile([P, F, 4], F32, tag="td")
                    nc.vector.tensor_tensor(ta[:], d1, cb, OP.mult)
                    nc.vector.tensor_tensor(tb2[:], d2, sb, OP.mult)
                    nc.vector.tensor_tensor(tcc[:], d2, cb, OP.mult)
                    nc.vector.tensor_tensor(td[:], d1, sb, OP.mult)
                    nc.vector.tensor_tensor(d1, ta[:], tb2[:], OP.add)
                    nc.vector.tensor_tensor(d2, tcc[:], td[:], OP.subtract)
                    k = sidx + 1
                    nc.scalar.copy(
                        feat[:, :, k * 12 : (k + 1) * 12],
                        states[:, :, k, 0:12],
                    )

                # ---- frames out ----
                nc.sync.dma_start(
                    frames[r0 : r0 + PF, :].rearrange("(p f) c -> p f c", f=F),
                    states[:].rearrange("p f k c -> p f (k c)"),
                )

                # ---- atoms ----
                xyzout = bpool.tile([P, F, 102], F32, tag="xyzout")
                for b in range(nblk):
                    nf = min(4, F - b * 4)
                    nb = nf * 128
                    tb = ppool.tile([108, 512], F32, tag="tb")
                    for fi in range(nf):
                        f_abs = b * 4 + fi
                        nc.tensor.transpose(
                            tb[:, fi * 128 : (fi + 1) * 128],
                            feat[:, f_abs, :],
                            iden_sb[:],
                        )
                    tsb = kpool.tile([108, 512], F32, tag="tsb")
                    nc.scalar.copy(tsb[:, :nb], tb[:, :nb])
                    xy = ppool.tile([102, 512], F32, tag="xy")
                    for s in range(5):
                        ms = ppool2.tile([108, 512], F32, tag="ms")
                        nc.tensor.matmul(
                            ms[:, :nb],
                            mskw_sb[:, s * 108 : (s + 1) * 108],
                            oht[:, b * 512 : b * 512 + nb],
                            start=True,
                            stop=True,
                        )
                        ph = kpool.tile([108, 512], F32, tag="ph")
                        nc.vector.tensor_tensor(
                            ph[:, :nb], tsb[:, :nb], ms[:, :nb], OP.mult
                        )
                        nc.tensor.matmul(
                            xy[:, :nb],
                            wat_sb[:, s * 102 : (s + 1) * 102],
                            ph[:, :nb],
                            start=(s == 0),
                            stop=(s == 4),
                        )
                    xys = kpool.tile([102, 512], F32, tag="xys")
                    nc.scalar.copy(xys[:, :nb], xy[:, :nb])
                    for fi in range(nf):
                        f_abs = b * 4 + fi
                        xt = ppool.tile([128, 102], F32, tag="xt")
                        nc.tensor.transpose(
                            xt[:],
                            xys[:, fi * 128 : (fi + 1) * 128],
                            iden_sb[0:102, 0:102],
                        )
                        nc.scalar.copy(xyzout[:, f_abs, :], xt[:])

                nc.sync.dma_start(
                    xyzo[r0 : r0 + PF, :].rearrange("(p f) c -> p f c", f=F),
                    xyzout[:],
                )

    nc.finalize()
    return nc


def _host_consts(xyzs_in_base_frame, RTs_in_base_frame, base_indices):
    xb = np.asarray(xyzs_in_base_frame, dtype=np.float32)      # (5,34,4)
    rts = np.asarray(RTs_in_base_frame, dtype=np.float32)      # (5,10,4,4)
    bi = np.asarray(base_indices).astype(np.int64)             # (5,34)

    rtt = np.zeros((5, 128), np.float32)
    for k2 in range(8):
        rtt[:, k2 * 16 : (k2 + 1) * 16] = rts[:, k2 + 2].reshape(5, 16)

    wat = np.zeros((108, 510), np.float32)
    for s in range(5):
        for a in range(34):
            k = int(bi[s, a])
            for i in range(3):
                for j in range(4):
                    wat[k * 12 + i * 4 + j, s * 102 + a * 3 + i] = xb[s, a, j]

    mskw = np.zeros((5, 540), np.float32)
    for s in range(5):
        mskw[s, s * 108 : (s + 1) * 108] = 1.0

    iden = np.eye(128, dtype=np.float32)
    iot5 = np.arange(5, dtype=np.float32).reshape(5, 1)
    return rtt, wat, mskw, iden, iot5


def prepare_in_maps(alphas, listRs, listTs, xyzs_in_base_frame,
                    RTs_in_base_frame, seq, base_indices):
    al = np.asarray(alphas, dtype=np.float32).reshape(L, 10, 2)[:, 2:10, :]
    al = np.ascontiguousarray(al).reshape(L, 16)
    rs = np.asarray(listRs, dtype=np.float32).reshape(L, 9)
    ts = np.asarray(listTs, dtype=np.float32).reshape(L, 3)
    rst = np.concatenate([rs, ts], axis=1)                     # (L,12)
    sq = np.asarray(seq).astype(np.float32)                    # (L,)

    rtt, wat, mskw, iden, iot5 = _host_consts(
        xyzs_in_base_frame, RTs_in_base_frame, base_indices
    )

    in_maps = []
    for c in range(NC):
        lo = c * LCORE
        a_pad = np.zeros((LCORE_PAD, 16), np.float32)
        a_pad[:LCORE] = al[lo : lo + LCORE]
        r_pad = np.zeros((LCORE_PAD, 12), np.float32)
        r_pad[:LCORE] = rst[lo : lo + LCORE]
        s_pad = np.zeros((LCORE_PAD,), np.float32)
        s_pad[:LCORE] = sq[lo : lo + LCORE]
        # seqp[t, :, f*128 + p] = seq[t*PF + p*F + f], replicated on 5 rows
        sq1 = s_pad.reshape(NT, P, F).transpose(0, 2, 1).reshape(NT, 1, PF)
        seqp = np.broadcast_to(sq1, (NT, 5, PF)).copy()
        in_maps.append(
            dict(alph=a_pad, rst=r_pad, seqp=seqp, rtt=rtt, wat=wat,
                 mskw=mskw, iden=iden, iot5=iot5)
        )
    return in_maps


def _get_runner():
    """Cached jitted PJRT runner (same execution path run_bass_kernel_spmd
    takes under axon, but built once so repeated kernel() calls don't
    re-trace/re-compile)."""
    if "runner" in _CACHE:
        return _CACHE["runner"]
    import jax
    import jax.numpy as jnp
    from jax.sharding import Mesh, PartitionSpec, NamedSharding
    from jax.experimental.shard_map import shard_map
    from concourse.bass2jax import (
        _bass_exec_p, install_neuronx_cc_hook, partition_id_tensor,
    )

    if "nc" not in _CACHE:
        _CACHE["nc"] = _build_bass()
    nc = _CACHE["nc"]
    install_neuronx_cc_hook()
    partition_name = nc.partition_id_tensor.name if nc.partition_id_tensor else None
    in_names, out_names, out_avals = [], [], []
    for alloc in nc.m.functions[0].allocations:
        if not isinstance(alloc, mybir.MemoryLocationSet):
            continue
        name = alloc.memorylocations[0].name
        if alloc.kind == "ExternalInput":
            if name != partition_name:
                in_names.append(name)
        elif alloc.kind == "ExternalOutput":
            out_names.append(name)
            out_avals.append(
                jax.core.ShapedArray(
                    tuple(alloc.tensor_shape), mybir.dt.np(alloc.dtype)
                )
            )
    in_names_all = list(in_names) + list(out_names)
    if partition_name is not None:
        in_names_all.append(partition_name)
    n_params = len(in_names)
    n_outs = len(out_avals)

    def _body(*args):
        operands = list(args)
        if partition_name is not None:
            operands.append(partition_id_tensor())
        outs = _bass_exec_p.bind(
            *operands,
            out_avals=tuple(out_avals),
            in_names=tuple(in_names_all),
            out_names=tuple(out_names),
            lowering_input_output_aliases=(),
            sim_require_finite=True,
            sim_require_nnan=True,
            nc=nc,
        )
        return tuple(outs)

    devices = jax.devices()[:NC]
    mesh = Mesh(np.asarray(devices), ("core",))
    shard = NamedSharding(mesh, PartitionSpec("core"))
    sharded = jax.jit(
        shard_map(
            _body, mesh=mesh,
            in_specs=(PartitionSpec("core"),) * (n_params + n_outs),
            out_specs=(PartitionSpec("core"),) * n_outs,
            check_rep=False,
        ),
        donate_argnums=tuple(range(n_params, n_params + n_outs)),
        keep_unused=True,
    )
    zero_shapes = [(NC * a.shape[0], *a.shape[1:]) for a in out_avals]
    zero_dtypes = [a.dtype for a in out_avals]
    mkz = jax.jit(
        lambda: tuple(jnp.zeros(s, d) for s, d in zip(zero_shapes, zero_dtypes)),
        out_shardings=tuple(shard for _ in zero_shapes),
    )

    def runner(in_maps):
        import jax as _jax
        concat_in = [
            _jax.device_put(
                np.concatenate(
                    [np.asarray(in_maps[c][nm]) for c in range(NC)], axis=0
                ),
                shard,
            )
            for nm in in_names
        ]
        outs = sharded(*concat_in, *mkz())
        _jax.block_until_ready(outs)
        per_core = []
        for c in range(NC):
            d = {}
            for i, nm in enumerate(out_names):
                d[nm] = np.asarray(outs[i]).reshape(NC, *out_avals[i].shape)[c]
            per_core.append(d)
        return per_core

    _CACHE["runner"] = runner
    return runner


class _Res:
    def __init__(self, results):
        self.results = results
        self.exec_time_ns = None


def run(in_maps, trace=False):
    try:
        return _Res(_get_runner()(in_maps))
    except Exception:
        if "nc" not in _CACHE:
            _CACHE["nc"] = _build_bass()
        return run_bass_kernel_spmd(
            _CACHE["nc"], in_maps, core_ids=list(range(NC)), trace=trace
        )


def kernel(alphas, listRs, listTs, xyzs_in_base_frame, RTs_in_base_frame,
           seq, base_indices):
    in_maps = prepare_in_maps(
        alphas, listRs, listTs, xyzs_in_base_frame, RTs_in_base_frame,
        seq, base_indices,
    )
    res = run(in_maps).results

    fr = np.concatenate(
        [r["frames"][:LCORE].reshape(LCORE, 9, 4, 4) for r in res], axis=0
    )
    xy = np.concatenate(
        [r["xyzo"][:LCORE].reshape(LCORE, 34, 3) for r in res], axis=0
    )
    return fr[None], xy[None]


# revision 22
# speedup vs baseline: 100.0782x; 71.5939x over previous
"""Trainium2 Bass kernel for ComputeAllAtomCoords (nn_ComputeAllAtomCoords).

Full inputs in, full outputs out. Internally: data-parallel over residues L
across 8 NeuronCores (sharding hint). Per core:
  - residues laid out SoA: residue = (partition p, free col f), l = p*F + f
  - torsion rotX normalization on DVE+ACT
  - per-residue RT matrices gathered from the 5-entry table via one-hot
    PE matmul (+PE transpose back to SoA layout)
  - frame chain RTF_k = parent @ RT_k @ rotX_k on DVE with broadcast-AP
    fused tensor_tensor ops
  - atom coords via 5 seq-masked PE matmuls with constant weights
    (xb points and base-frame indices baked into the weights host-side)
"""

import sys

sys.path.insert(0, "/opt/trn_rl_repo")

import numpy as np

import concourse.bass as bass
import concourse.bacc as bacc
import concourse.mybir as mybir
from concourse.bass_utils import run_bass_kernel_spmd
from concourse.tile import TileContext

F32 = mybir.dt.float32
OP = mybir.AluOpType

P = 128          # partitions
F = 49           # free columns per tile
NT = 4           # tiles per core
PF = P * F       # residues per tile (6272)
LCORE_PAD = PF * NT   # 25088
L = 200000
NC = 8
LCORE = L // NC  # 25000
EPS = 1e-6

# chain: frames 1..8 = step(parent_state, torsion k=sidx+2)
PARENT = [0, 1, 2, 3, 3, 5, 6, 6]

_CACHE = {}


def _build_bass():
    nc = bacc.Bacc()

    # DRAM I/O (per-core shapes)
    alph = nc.dram_tensor("alph", [LCORE_PAD, 16], F32, kind="ExternalInput")
    rst = nc.dram_tensor("rst", [LCORE_PAD, 12], F32, kind="ExternalInput")
    seqp = nc.dram_tensor("seqp", [NT, 5, PF], F32, kind="ExternalInput")
    rtt = nc.dram_tensor("rtt", [5, 128], F32, kind="ExternalInput")
    wat = nc.dram_tensor("wat", [108, 510], F32, kind="ExternalInput")
    mskw = nc.dram_tensor("mskw", [5, 540], F32, kind="ExternalInput")
    iden = nc.dram_tensor("iden", [128, 128], F32, kind="ExternalInput")
    iot5 = nc.dram_tensor("iot5", [5, 1], F32, kind="ExternalInput")
    frames = nc.dram_tensor("frames", [LCORE_PAD, 144], F32, kind="ExternalOutput")
    xyzo = nc.dram_tensor("xyzo", [LCORE_PAD, 102], F32, kind="ExternalOutput")

    with TileContext(nc) as tc:
        with (
            tc.tile_pool(name="const", bufs=1) as cpool,
            tc.tile_pool(name="big", bufs=1) as bpool,
            tc.tile_pool(name="ld", bufs=2) as lpool,
            tc.tile_pool(name="small", bufs=2) as spool,
            tc.tile_pool(name="blk", bufs=2) as kpool,
            tc.tile_pool(name="ft", bufs=2) as fpool,
            tc.tile_pool(name="psum", bufs=1, space="PSUM") as ppool,
            tc.tile_pool(name="psum2", bufs=2, space="PSUM") as ppool2,
        )            :
            # --- constants into SBUF ---
            rtt_sb = cpool.tile([5, 128], F32, tag="rtt")
            nc.sync.dma_start(rtt_sb[:], rtt[:, :])
            wat_sb = cpool.tile([108, 510], F32, tag="wat")
            nc.sync.dma_start(wat_sb[:], wat[:, :])
            mskw_sb = cpool.tile([5, 540], F32, tag="mskw")
            nc.sync.dma_start(mskw_sb[:], mskw[:, :])
            iden_sb = cpool.tile([128, 128], F32, tag="iden")
            nc.sync.dma_start(iden_sb[:], iden[:, :])
            iot5_sb = cpool.tile([5, 1], F32, tag="iot5")
            nc.sync.dma_start(iot5_sb[:], iot5[:, :])

            for t in range(NT):
                r0 = t * PF
                # ---- loads ----
                ang = lpool.tile([P, F, 16], F32, tag="ang")
                nc.sync.dma_start(
                    ang[:],
                    alph[r0 : r0 + PF, :].rearrange("(p f) c -> p f c", f=F),
                )
                # states: (P, F, 9, 16) — frame k comps contiguous per residue
                states = bpool.tile([P, F, 9, 16], F32, tag="states")
                rstg = lpool.tile([P, F, 12], F32, tag="rstg")
                nc.sync.dma_start(
                    rstg[:], rst[r0 : r0 + PF, :].rearrange("(p f) c -> p f c", f=F)
                )
                st0r = states[:, :, 0, :].rearrange("p f (i m) -> p f i m", m=4)
                rr = rstg[:, :, 0:9].rearrange("p f (i j) -> p f i j", j=3)
                nc.vector.tensor_copy(st0r[:, :, 0:3, 0:3], rr)
                nc.vector.tensor_copy(st0r[:, :, 0:3, 3], rstg[:, :, 9:12])
                nc.gpsimd.memset(states[:, :, 0, 12:15], 0.0)
                nc.gpsimd.memset(states[:, :, 0, 15:16], 1.0)

                oht = bpool.tile([5, PF], F32, tag="oht")
                nc.sync.dma_start(oht[:], seqp[t])

                # ---- rotX normalization ----
                c_ap = ang[:, :, 0:16:2]
                s_ap = ang[:, :, 1:16:2]
                sq = spool.tile([P, F, 8], F32, tag="sq")
                n2 = spool.tile([P, F, 8], F32, tag="n2")
                rec = spool.tile([P, F, 8], F32, tag="rec")
                nc.vector.tensor_tensor(sq[:], s_ap, s_ap, OP.mult)
                nc.vector.tensor_tensor(n2[:], c_ap, c_ap, OP.mult)
                nc.vector.tensor_tensor(n2[:], n2[:], sq[:], OP.add)
                nc.scalar.sqrt(sq[:], n2[:])
                nc.vector.tensor_scalar_add(sq[:], sq[:], EPS)
                nc.vector.reciprocal(rec[:], sq[:])
                angn = bpool.tile([P, F, 16], F32, tag="angn")
                nc.vector.tensor_tensor(angn[:, :, 0:16:2], c_ap, rec[:], OP.mult)
                nc.vector.tensor_tensor(angn[:, :, 1:16:2], s_ap, rec[:], OP.mult)

                # ---- one-hot over seq (5, PF), in place ----
                nc.vector.tensor_scalar(
                    oht[:], oht[:], iot5_sb[:, 0:1], None, OP.is_equal
                )

                # ---- RTbar gather: (5->128) matmul + transpose to SoA ----
                rtbar = bpool.tile([P, F, 8, 16], F32, tag="rtbar")
                nblk = (F + 3) // 4
                for b in range(nblk):
                    nf = min(4, F - b * 4)
                    nb = nf * 128
                    rb = ppool.tile([128, 512], F32, tag="rb")
                    nc.tensor.matmul(
                        rb[:, :nb],
                        rtt_sb[:],
                        oht[:, b * 512 : b * 512 + nb],
                        start=True,
                        stop=True,
                    )
                    rbs = kpool.tile([128, 512], F32, tag="rbs")
                    nc.scalar.copy(rbs[:, :nb], rb[:, :nb])
                    for fi in range(nf):
                        f_abs = b * 4 + fi
                        tp = ppool.tile([128, 128], F32, tag="tp")
                        nc.tensor.transpose(
                            tp[:], rbs[:, fi * 128 : (fi + 1) * 128], iden_sb[:]
                        )
                        nc.scalar.copy(
                            rtbar[:, f_abs],
                            tp[:].rearrange("p (k c) -> p k c", c=16),
                        )

                # ---- chain ----
                # feat: contiguous rows 0-2 of every frame, for the PE bridge
                feat = fpool.tile([P, F, 108], F32, tag="feat")
                ftr = feat[:, :, 0:12].rearrange("p f (i j) -> p f i j", j=4)
                nc.scalar.copy(ftr[:, :, :, 0:3], rr)
                nc.scalar.copy(ftr[:, :, :, 3], rstg[:, :, 9:12])
                for sidx in range(8):
                    Pst = states[:, :, PARENT[sidx]].rearrange(
                        "p f (i m) -> p f i m", m=4
                    )
                    C4 = states[:, :, sidx + 1].rearrange("p f (i m) -> p f i m", m=4)
                    B4 = rtbar[:, :, sidx, :].rearrange("p f (j m) -> p f j m", m=4)
                    tmp4a = spool.tile([P, F, 4, 4], F32, tag="tmp4a")
                    tmp4b = spool.tile([P, F, 4, 4], F32, tag="tmp4b")
                    for j in range(4):
                        pjb = Pst[:, :, :, j].unsqueeze(3).broadcast_to([P, F, 4, 4])
                        bjb = B4[:, :, j, :].unsqueeze(2).broadcast_to([P, F, 4, 4])
                        if j == 0:
                            nc.vector.tensor_tensor(C4[:], pjb, bjb, OP.mult)
                        else:
                            tmp4 = tmp4a if j % 2 else tmp4b
                            nc.vector.tensor_tensor(tmp4[:], pjb, bjb, OP.mult)
                            nc.gpsimd.tensor_tensor(C4[:], C4[:], tmp4[:], OP.add)
                    # right-multiply by rotX(sidx): cols 1,2 mix
                    cb = angn[:, :, 2 * sidx].unsqueeze(2).broadcast_to([P, F, 4])
                    sb = angn[:, :, 2 * sidx + 1].unsqueeze(2).broadcast_to([P, F, 4])
                    d1 = C4[:, :, :, 1]
                    d2 = C4[:, :, :, 2]
                    ta = spool.tile([P, F, 4], F32, tag="ta")
                    tb2 = spool.tile([P, F, 4], F32, tag="tb2")
                    tcc = spool.tile([P, F, 4], F32, tag="tcc")
                    td = spool.tile([P, F, 4], F32, tag="td")
                    nc.vector.tensor_tensor(ta[:], d1, cb, OP.mult)
                    nc.vector.tensor_tensor(tb2[:], d2, sb, OP.mult)
                    nc.vector.tensor_tensor(tcc[:], d2, cb, OP.mult)
                    nc.vector.tensor_tensor(td[:], d1, sb, OP.mult)
                    nc.vector.tensor_tensor(d1, ta[:], tb2[:], OP.add)
                    nc.vector.tensor_tensor(d2, tcc[:], td[:], OP.subtract)
                    k = sidx + 1
                    nc.scalar.copy(
                        feat[:, :, k * 12 : (k + 1) * 12],
                        states[:, :, k, 0:12],
                    )

                # ---- frames out ----
                nc.sync.dma_start(
                    frames[r0 : r0 + PF, :].rearrange("(p f) c -> p f c", f=F),
                    states[:].rearrange("p f k c -> p f (k c)"),
                )

                # ---- atoms ----
                xyzout = bpool.tile([P, F, 102], F32, tag="xyzout")
                for b in range(nblk):
                    nf = min(4, F - b * 4)
                    nb = nf * 128
                    tb = ppool.tile([108, 512], F32, tag="tb")
                    for fi in range(nf):
                        f_abs = b * 4 + fi
                        nc.tensor.transpose(
                            tb[:, fi * 128 : (fi + 1) * 128],
                            feat[:, f_abs, :],
                            iden_sb[:],
                        )
                    tsb = kpool.tile([108, 512], F32, tag="tsb")
                    nc.scalar.copy(tsb[:, :nb], tb[:, :nb])
                    xy = ppool.tile([102, 512], F32, tag="xy")
                    for s in range(5):
                        ms = ppool2.tile([108, 512], F32, tag="ms")
                        nc.tensor.matmul(
                            ms[:, :nb],
                            mskw_sb[:, s * 108 : (s + 1) * 108],
                            oht[:, b * 512 : b * 512 + nb],
                            start=True,
                            stop=True,
                        )
                        ph = kpool.tile([108, 512], F32, tag="ph")
                        nc.vector.tensor_tensor(
                            ph[:, :nb], tsb[:, :nb], ms[:, :nb], OP.mult
                        )
                        nc.tensor.matmul(
                            xy[:, :nb],
                            wat_sb[:, s * 102 : (s + 1) * 102],
                            ph[:, :nb],
                            start=(s == 0),
                            stop=(s == 4),
                        )
                    xys = kpool.tile([102, 512], F32, tag="xys")
                    nc.scalar.copy(xys[:, :nb], xy[:, :nb])
                    for fi in range(nf):
                        f_abs = b * 4 + fi
                        xt = ppool.tile([128, 102], F32, tag="xt")
                        nc.tensor.transpose(
                            xt[:],
                            xys[:, fi * 128 : (fi + 1) * 128],
                            iden_sb[0:102, 0:102],
                        )
                        nc.scalar.copy(xyzout[:, f_abs, :], xt[:])

                nc.sync.dma_start(
                    xyzo[r0 : r0 + PF, :].rearrange("(p f) c -> p f c", f=F),
                    xyzout[:],
                )

    nc.finalize()
    return nc


def _host_consts(xyzs_in_base_frame, RTs_in_base_frame, base_indices):
    xb = np.asarray(xyzs_in_base_frame, dtype=np.float32)      # (5,34,4)
    rts = np.asarray(RTs_in_base_frame, dtype=np.float32)      # (5,10,4,4)
    bi = np.asarray(base_indices).astype(np.int64)             # (5,34)

    rtt = np.zeros((5, 128), np.float32)
    for k2 in range(8):
        rtt[:, k2 * 16 : (k2 + 1) * 16] = rts[:, k2 + 2].reshape(5, 16)

    wat = np.zeros((108, 510), np.float32)
    for s in range(5):
        for a in range(34):
            k = int(bi[s, a])
            for i in range(3):
                for j in range(4):
                    wat[k * 12 + i * 4 + j, s * 102 + a * 3 + i] = xb[s, a, j]

    mskw = np.zeros((5, 540), np.float32)
    for s in range(5):
        mskw[s, s * 108 : (s + 1) * 108] = 1.0

    iden = np.eye(128, dtype=np.float32)
    iot5 = np.arange(5, dtype=np.float32).reshape(5, 1)
    return rtt, wat, mskw, iden, iot5


def prepare_in_maps(alphas, listRs, listTs, xyzs_in_base_frame,
                    RTs_in_base_frame, seq, base_indices):
    al = np.asarray(alphas, dtype=np.float32).reshape(L, 10, 2)[:, 2:10, :]
    al = np.ascontiguousarray(al).reshape(L, 16)
    rs = np.asarray(listRs, dtype=np.float32).reshape(L, 9)
    ts = np.asarray(listTs, dtype=np.float32).reshape(L, 3)
    rst = np.concatenate([rs, ts], axis=1)                     # (L,12)
    sq = np.asarray(seq).astype(np.float32)                    # (L,)

    rtt, wat, mskw, iden, iot5 = _host_consts(
        xyzs_in_base_frame, RTs_in_base_frame, base_indices
    )

    in_maps = []
    for c in range(NC):
        lo = c * LCORE
        a_pad = np.zeros((LCORE_PAD, 16), np.float32)
        a_pad[:LCORE] = al[lo : lo + LCORE]
        r_pad = np.zeros((LCORE_PAD, 12), np.float32)
        r_pad[:LCORE] = rst[lo : lo + LCORE]
        s_pad = np.zeros((LCORE_PAD,), np.float32)
        s_pad[:LCORE] = sq[lo : lo + LCORE]
        # seqp[t, :, f*128 + p] = seq[t*PF + p*F + f], replicated on 5 rows
        sq1 = s_pad.reshape(NT, P, F).transpose(0, 2, 1).reshape(NT, 1, PF)
        seqp = np.broadcast_to(sq1, (NT, 5, PF)).copy()
        in_maps.append(
            dict(alph=a_pad, rst=r_pad, seqp=seqp, rtt=rtt, wat=wat,
                 mskw=mskw, iden=iden, iot5=iot5)
        )
    return in_maps


def _get_runner():
    """Cached jitted PJRT runner (same execution path run_bass_kernel_spmd
    takes under axon, but built once so repeated kernel() calls don't
    re-trace/re-compile)."""
    if "runner" in _CACHE:
        return _CACHE["runner"]
    import jax
    import jax.numpy as jnp
    from jax.sharding import Mesh, PartitionSpec, NamedSharding
    from jax.experimental.shard_map import shard_map
    from concourse.bass2jax import (
        _bass_exec_p, install_neuronx_cc_hook, partition_id_tensor,
    )

    if "nc" not in _CACHE:
        _CACHE["nc"] = _build_bass()
    nc = _CACHE["nc"]
    install_neuronx_cc_hook()
    partition_name = nc.partition_id_tensor.name if nc.partition_id_tensor else None
    in_names, out_names, out_avals = [], [], []
    for alloc in nc.m.functions[0].allocations:
        if not isinstance(alloc, mybir.MemoryLocationSet):
            continue
        name = alloc.memorylocations[0].name
        if alloc.kind == "ExternalInput":
            if name != partition_name:
                in_names.append(name)
        elif alloc.kind == "ExternalOutput":
            out_names.append(name)
            out_avals.append(
                jax.core.ShapedArray(
                    tuple(alloc.tensor_shape), mybir.dt.np(alloc.dtype)
                )
            )
    in_names_all = list(in_names) + list(out_names)
    if partition_name is not None:
        in_names_all.append(partition_name)
    n_params = len(in_names)
    n_outs = len(out_avals)

    def _body(*args):
        operands = list(args)
        if partition_name is not None:
            operands.append(partition_id_tensor())
        outs = _bass_exec_p.bind(
            *operands,
            out_avals=tuple(out_avals),
            in_names=tuple(in_names_all),
            out_names=tuple(out_names),
            lowering_input_output_aliases=(),
            sim_require_finite=True,
            sim_require_nnan=True,
            nc=nc,
        )
        return tuple(outs)

    devices = jax.devices()[:NC]
    mesh = Mesh(np.asarray(devices), ("core",))
    shard = NamedSharding(mesh, PartitionSpec("core"))
    sharded = jax.jit(
        shard_map(
            _body, mesh=mesh,
            in_specs=(PartitionSpec("core"),) * (n_params + n_outs),
            out_specs=(PartitionSpec("core"),) * n_outs,
            check_rep=False,
        ),
        donate_argnums=tuple(range(n_params, n_params + n_outs)),
        keep_unused=True,
    )
    zero_shapes = [(NC * a.shape[0], *a.shape[1:]) for a in out_avals]
    zero_dtypes = [a.dtype for a in out_avals]
    mkz = jax.jit(
        lambda: tuple(jnp.zeros(s, d) for s, d in zip(zero_shapes, zero_dtypes)),
        out_shardings=tuple(shard for _ in zero_shapes),
    )

    def runner(in_maps):
        import jax as _jax
        concat_in = [
            _jax.device_put(
                np.concatenate(
                    [np.asarray(in_maps[c][nm]) for c in range(NC)], axis=0
                ),
                shard,
            )
            for nm in in_names
        ]
        outs = sharded(*concat_in, *mkz())
        _jax.block_until_ready(outs)
        per_core = []
        for c in range(NC):
            d = {}
            for i, nm in enumerate(out_names):
                d[nm] = np.asarray(outs[i]).reshape(NC, *out_avals[i].shape)[c]
            per_core.append(d)
        return per_core

    _CACHE["runner"] = runner
    _CACHE["internals"] = dict(
        sharded=sharded, mkz=mkz, in_names=in_names, out_names=out_names,
        out_avals=out_avals, shard=shard,
    )
    return runner


class _Res:
    def __init__(self, results):
        self.results = results
        self.exec_time_ns = None


def run(in_maps, trace=False):
    try:
        return _Res(_get_runner()(in_maps))
    except Exception:
        if "nc" not in _CACHE:
            _CACHE["nc"] = _build_bass()
        return run_bass_kernel_spmd(
            _CACHE["nc"], in_maps, core_ids=list(range(NC)), trace=trace
        )


def kernel(alphas, listRs, listTs, xyzs_in_base_frame, RTs_in_base_frame,
           seq, base_indices):
    in_maps = prepare_in_maps(
        alphas, listRs, listTs, xyzs_in_base_frame, RTs_in_base_frame,
        seq, base_indices,
    )
    res = run(in_maps).results

    fr = np.concatenate(
        [r["frames"][:LCORE].reshape(LCORE, 9, 4, 4) for r in res], axis=0
    )
    xy = np.concatenate(
        [r["xyzo"][:LCORE].reshape(LCORE, 34, 3) for r in res], axis=0
    )
    return fr[None], xy[None]


# revision 23
# speedup vs baseline: 531.0041x; 5.3059x over previous
"""Trainium2 Bass kernel for ComputeAllAtomCoords (nn_ComputeAllAtomCoords).

Full inputs in, full outputs out. Internally: data-parallel over residues L
across 8 NeuronCores (sharding hint). Per core:
  - residues laid out SoA: residue = (partition p, free col f), l = p*F + f
  - torsion rotX normalization on DVE+ACT
  - per-residue RT matrices gathered from the 5-entry table via one-hot
    PE matmul (+PE transpose back to SoA layout)
  - frame chain RTF_k = parent @ RT_k @ rotX_k on DVE with broadcast-AP
    fused tensor_tensor ops
  - atom coords via 5 seq-masked PE matmuls with constant weights
    (xb points and base-frame indices baked into the weights host-side)
"""

import sys

sys.path.insert(0, "/opt/trn_rl_repo")

import numpy as np

import concourse.bass as bass
import concourse.bacc as bacc
import concourse.mybir as mybir
from concourse.bass_utils import run_bass_kernel_spmd
from concourse.tile import TileContext

F32 = mybir.dt.float32
OP = mybir.AluOpType

P = 128          # partitions
F = 49           # free columns per tile
NT = 4           # tiles per core
PF = P * F       # residues per tile (6272)
LCORE_PAD = PF * NT   # 25088
L = 200000
NC = 8
LCORE = L // NC  # 25000
EPS = 1e-6

# chain: frames 1..8 = step(parent_state, torsion k=sidx+2)
PARENT = [0, 1, 2, 3, 3, 5, 6, 6]

_CACHE = {}


def _build_bass():
    nc = bacc.Bacc()

    # DRAM I/O (per-core shapes)
    alph = nc.dram_tensor("alph", [LCORE_PAD, 16], F32, kind="ExternalInput")
    rst = nc.dram_tensor("rst", [LCORE_PAD, 12], F32, kind="ExternalInput")
    seqp = nc.dram_tensor("seqp", [NT, 5, PF], F32, kind="ExternalInput")
    rtt = nc.dram_tensor("rtt", [5, 128], F32, kind="ExternalInput")
    wat = nc.dram_tensor("wat", [108, 510], F32, kind="ExternalInput")
    mskw = nc.dram_tensor("mskw", [5, 540], F32, kind="ExternalInput")
    iden = nc.dram_tensor("iden", [128, 128], F32, kind="ExternalInput")
    iot5 = nc.dram_tensor("iot5", [5, 1], F32, kind="ExternalInput")
    frames = nc.dram_tensor("frames", [LCORE_PAD, 144], F32, kind="ExternalOutput")
    xyzo = nc.dram_tensor("xyzo", [LCORE_PAD, 102], F32, kind="ExternalOutput")

    with TileContext(nc) as tc:
        with (
            tc.tile_pool(name="const", bufs=1) as cpool,
            tc.tile_pool(name="big", bufs=1) as bpool,
            tc.tile_pool(name="ld", bufs=2) as lpool,
            tc.tile_pool(name="small", bufs=2) as spool,
            tc.tile_pool(name="blk", bufs=2) as kpool,
            tc.tile_pool(name="ft", bufs=2) as fpool,
            tc.tile_pool(name="psum", bufs=1, space="PSUM") as ppool,
            tc.tile_pool(name="psum2", bufs=2, space="PSUM") as ppool2,
        )            :
            # --- constants into SBUF ---
            rtt_sb = cpool.tile([5, 128], F32, tag="rtt")
            nc.sync.dma_start(rtt_sb[:], rtt[:, :])
            wat_sb = cpool.tile([108, 510], F32, tag="wat")
            nc.sync.dma_start(wat_sb[:], wat[:, :])
            mskw_sb = cpool.tile([5, 540], F32, tag="mskw")
            nc.sync.dma_start(mskw_sb[:], mskw[:, :])
            iden_sb = cpool.tile([128, 128], F32, tag="iden")
            nc.sync.dma_start(iden_sb[:], iden[:, :])
            iot5_sb = cpool.tile([5, 1], F32, tag="iot5")
            nc.sync.dma_start(iot5_sb[:], iot5[:, :])

            for t in range(NT):
                r0 = t * PF
                # ---- loads ----
                ang = lpool.tile([P, F, 16], F32, tag="ang")
                nc.sync.dma_start(
                    ang[:],
                    alph[r0 : r0 + PF, :].rearrange("(p f) c -> p f c", f=F),
                )
                # states: (P, F, 9, 16) — frame k comps contiguous per residue
                states = bpool.tile([P, F, 9, 16], F32, tag="states")
                rstg = lpool.tile([P, F, 12], F32, tag="rstg")
                nc.sync.dma_start(
                    rstg[:], rst[r0 : r0 + PF, :].rearrange("(p f) c -> p f c", f=F)
                )
                st0r = states[:, :, 0, :].rearrange("p f (i m) -> p f i m", m=4)
                rr = rstg[:, :, 0:9].rearrange("p f (i j) -> p f i j", j=3)
                nc.vector.tensor_copy(st0r[:, :, 0:3, 0:3], rr)
                nc.vector.tensor_copy(st0r[:, :, 0:3, 3], rstg[:, :, 9:12])
                nc.gpsimd.memset(states[:, :, 0, 12:15], 0.0)
                nc.gpsimd.memset(states[:, :, 0, 15:16], 1.0)

                oht = bpool.tile([5, PF], F32, tag="oht")
                nc.sync.dma_start(oht[:], seqp[t])

                # ---- rotX normalization ----
                c_ap = ang[:, :, 0:16:2]
                s_ap = ang[:, :, 1:16:2]
                sq = spool.tile([P, F, 8], F32, tag="sq")
                n2 = spool.tile([P, F, 8], F32, tag="n2")
                rec = spool.tile([P, F, 8], F32, tag="rec")
                nc.vector.tensor_tensor(sq[:], s_ap, s_ap, OP.mult)
                nc.vector.tensor_tensor(n2[:], c_ap, c_ap, OP.mult)
                nc.vector.tensor_tensor(n2[:], n2[:], sq[:], OP.add)
                nc.scalar.sqrt(sq[:], n2[:])
                nc.vector.tensor_scalar_add(sq[:], sq[:], EPS)
                nc.vector.reciprocal(rec[:], sq[:])
                angn = bpool.tile([P, F, 16], F32, tag="angn")
                nc.vector.tensor_tensor(angn[:, :, 0:16:2], c_ap, rec[:], OP.mult)
                nc.vector.tensor_tensor(angn[:, :, 1:16:2], s_ap, rec[:], OP.mult)

                # ---- one-hot over seq (5, PF), in place ----
                nc.vector.tensor_scalar(
                    oht[:], oht[:], iot5_sb[:, 0:1], None, OP.is_equal
                )

                # ---- RTbar gather: (5->128) matmul + transpose to SoA ----
                rtbar = bpool.tile([P, F, 8, 16], F32, tag="rtbar")
                nblk = (F + 3) // 4
                for b in range(nblk):
                    nf = min(4, F - b * 4)
                    nb = nf * 128
                    rb = ppool.tile([128, 512], F32, tag="rb")
                    nc.tensor.matmul(
                        rb[:, :nb],
                        rtt_sb[:],
                        oht[:, b * 512 : b * 512 + nb],
                        start=True,
                        stop=True,
                    )
                    rbs = kpool.tile([128, 512], F32, tag="rbs")
                    nc.scalar.copy(rbs[:, :nb], rb[:, :nb])
                    for fi in range(nf):
                        f_abs = b * 4 + fi
                        tp = ppool.tile([128, 128], F32, tag="tp")
                        nc.tensor.transpose(
                            tp[:], rbs[:, fi * 128 : (fi + 1) * 128], iden_sb[:]
                        )
                        nc.scalar.copy(
                            rtbar[:, f_abs],
                            tp[:].rearrange("p (k c) -> p k c", c=16),
                        )

                # ---- chain ----
                # feat: contiguous rows 0-2 of every frame, for the PE bridge
                feat = fpool.tile([P, F, 108], F32, tag="feat")
                ftr = feat[:, :, 0:12].rearrange("p f (i j) -> p f i j", j=4)
                nc.scalar.copy(ftr[:, :, :, 0:3], rr)
                nc.scalar.copy(ftr[:, :, :, 3], rstg[:, :, 9:12])
                for sidx in range(8):
                    Pst = states[:, :, PARENT[sidx]].rearrange(
                        "p f (i m) -> p f i m", m=4
                    )
                    C4 = states[:, :, sidx + 1].rearrange("p f (i m) -> p f i m", m=4)
                    B4 = rtbar[:, :, sidx, :].rearrange("p f (j m) -> p f j m", m=4)
                    tmp4a = spool.tile([P, F, 4, 4], F32, tag="tmp4a")
                    tmp4b = spool.tile([P, F, 4, 4], F32, tag="tmp4b")
                    for j in range(4):
                        pjb = Pst[:, :, :, j].unsqueeze(3).broadcast_to([P, F, 4, 4])
                        bjb = B4[:, :, j, :].unsqueeze(2).broadcast_to([P, F, 4, 4])
                        if j == 0:
                            nc.vector.tensor_tensor(C4[:], pjb, bjb, OP.mult)
                        else:
                            tmp4 = tmp4a if j % 2 else tmp4b
                            nc.vector.tensor_tensor(tmp4[:], pjb, bjb, OP.mult)
                            nc.vector.tensor_tensor(C4[:], C4[:], tmp4[:], OP.add)
                    # right-multiply by rotX(sidx): cols 1,2 mix
                    cb = angn[:, :, 2 * sidx].unsqueeze(2).broadcast_to([P, F, 4])
                    sb = angn[:, :, 2 * sidx + 1].unsqueeze(2).broadcast_to([P, F, 4])
                    d1 = C4[:, :, :, 1]
                    d2 = C4[:, :, :, 2]
                    ta = spool.tile([P, F, 4], F32, tag="ta")
                    tb2 = spool.tile([P, F, 4], F32, tag="tb2")
                    tcc = spool.tile([P, F, 4], F32, tag="tcc")
                    td = spool.tile([P, F, 4], F32, tag="td")
                    nc.vector.tensor_tensor(ta[:], d1, cb, OP.mult)
                    nc.vector.tensor_tensor(tb2[:], d2, sb, OP.mult)
                    nc.vector.tensor_tensor(tcc[:], d2, cb, OP.mult)
                    nc.vector.tensor_tensor(td[:], d1, sb, OP.mult)
                    nc.vector.tensor_tensor(d1, ta[:], tb2[:], OP.add)
                    nc.vector.tensor_tensor(d2, tcc[:], td[:], OP.subtract)
                    k = sidx + 1
                    nc.scalar.copy(
                        feat[:, :, k * 12 : (k + 1) * 12],
                        states[:, :, k, 0:12],
                    )

                # ---- frames out ----
                nc.sync.dma_start(
                    frames[r0 : r0 + PF, :].rearrange("(p f) c -> p f c", f=F),
                    states[:].rearrange("p f k c -> p f (k c)"),
                )

                # ---- atoms ----
                xyzout = bpool.tile([P, F, 102], F32, tag="xyzout")
                for b in range(nblk):
                    nf = min(4, F - b * 4)
                    nb = nf * 128
                    tb = ppool.tile([108, 512], F32, tag="tb")
                    for fi in range(nf):
                        f_abs = b * 4 + fi
                        nc.tensor.transpose(
                            tb[:, fi * 128 : (fi + 1) * 128],
                            feat[:, f_abs, :],
                            iden_sb[:],
                        )
                    tsb = kpool.tile([108, 512], F32, tag="tsb")
                    nc.scalar.copy(tsb[:, :nb], tb[:, :nb])
                    xy = ppool.tile([102, 512], F32, tag="xy")
                    for s in range(5):
                        ms = ppool2.tile([108, 512], F32, tag="ms")
                        nc.tensor.matmul(
                            ms[:, :nb],
                            mskw_sb[:, s * 108 : (s + 1) * 108],
                            oht[:, b * 512 : b * 512 + nb],
                            start=True,
                            stop=True,
                        )
                        ph = kpool.tile([108, 512], F32, tag="ph")
                        nc.vector.tensor_tensor(
                            ph[:, :nb], tsb[:, :nb], ms[:, :nb], OP.mult
                        )
                        nc.tensor.matmul(
                            xy[:, :nb],
                            wat_sb[:, s * 102 : (s + 1) * 102],
                            ph[:, :nb],
                            start=(s == 0),
                            stop=(s == 4),
                        )
                    xys = kpool.tile([102, 512], F32, tag="xys")
                    nc.scalar.copy(xys[:, :nb], xy[:, :nb])
                    for fi in range(nf):
                        f_abs = b * 4 + fi
                        xt = ppool.tile([128, 102], F32, tag="xt")
                        nc.tensor.transpose(
                            xt[:],
                            xys[:, fi * 128 : (fi + 1) * 128],
                            iden_sb[0:102, 0:102],
                        )
                        nc.scalar.copy(xyzout[:, f_abs, :], xt[:])

                nc.sync.dma_start(
                    xyzo[r0 : r0 + PF, :].rearrange("(p f) c -> p f c", f=F),
                    xyzout[:],
                )

    nc.finalize()
    return nc


def _host_consts(xyzs_in_base_frame, RTs_in_base_frame, base_indices):
    xb = np.asarray(xyzs_in_base_frame, dtype=np.float32)      # (5,34,4)
    rts = np.asarray(RTs_in_base_frame, dtype=np.float32)      # (5,10,4,4)
    bi = np.asarray(base_indices).astype(np.int64)             # (5,34)

    rtt = np.zeros((5, 128), np.float32)
    for k2 in range(8):
        rtt[:, k2 * 16 : (k2 + 1) * 16] = rts[:, k2 + 2].reshape(5, 16)

    wat = np.zeros((108, 510), np.float32)
    for s in range(5):
        for a in range(34):
            k = int(bi[s, a])
            for i in range(3):
                for j in range(4):
                    wat[k * 12 + i * 4 + j, s * 102 + a * 3 + i] = xb[s, a, j]

    mskw = np.zeros((5, 540), np.float32)
    for s in range(5):
        mskw[s, s * 108 : (s + 1) * 108] = 1.0

    iden = np.eye(128, dtype=np.float32)
    iot5 = np.arange(5, dtype=np.float32).reshape(5, 1)
    return rtt, wat, mskw, iden, iot5


def prepare_in_maps(alphas, listRs, listTs, xyzs_in_base_frame,
                    RTs_in_base_frame, seq, base_indices):
    al = np.asarray(alphas, dtype=np.float32).reshape(L, 10, 2)[:, 2:10, :]
    al = np.ascontiguousarray(al).reshape(L, 16)
    rs = np.asarray(listRs, dtype=np.float32).reshape(L, 9)
    ts = np.asarray(listTs, dtype=np.float32).reshape(L, 3)
    rst = np.concatenate([rs, ts], axis=1)                     # (L,12)
    sq = np.asarray(seq).astype(np.float32)                    # (L,)

    rtt, wat, mskw, iden, iot5 = _host_consts(
        xyzs_in_base_frame, RTs_in_base_frame, base_indices
    )

    in_maps = []
    for c in range(NC):
        lo = c * LCORE
        a_pad = np.zeros((LCORE_PAD, 16), np.float32)
        a_pad[:LCORE] = al[lo : lo + LCORE]
        r_pad = np.zeros((LCORE_PAD, 12), np.float32)
        r_pad[:LCORE] = rst[lo : lo + LCORE]
        s_pad = np.zeros((LCORE_PAD,), np.float32)
        s_pad[:LCORE] = sq[lo : lo + LCORE]
        # seqp[t, :, f*128 + p] = seq[t*PF + p*F + f], replicated on 5 rows
        sq1 = s_pad.reshape(NT, P, F).transpose(0, 2, 1).reshape(NT, 1, PF)
        seqp = np.broadcast_to(sq1, (NT, 5, PF)).copy()
        in_maps.append(
            dict(alph=a_pad, rst=r_pad, seqp=seqp, rtt=rtt, wat=wat,
                 mskw=mskw, iden=iden, iot5=iot5)
        )
    return in_maps


def _get_runner():
    """Cached jitted PJRT runner (same execution path run_bass_kernel_spmd
    takes under axon, but built once so repeated kernel() calls don't
    re-trace/re-compile)."""
    if "runner" in _CACHE:
        return _CACHE["runner"]
    import jax
    import jax.numpy as jnp
    from jax.sharding import Mesh, PartitionSpec, NamedSharding
    from jax.experimental.shard_map import shard_map
    from concourse.bass2jax import (
        _bass_exec_p, install_neuronx_cc_hook, partition_id_tensor,
    )

    if "nc" not in _CACHE:
        _CACHE["nc"] = _build_bass()
    nc = _CACHE["nc"]
    install_neuronx_cc_hook()
    partition_name = nc.partition_id_tensor.name if nc.partition_id_tensor else None
    in_names, out_names, out_avals = [], [], []
    for alloc in nc.m.functions[0].allocations:
        if not isinstance(alloc, mybir.MemoryLocationSet):
            continue
        name = alloc.memorylocations[0].name
        if alloc.kind == "ExternalInput":
            if name != partition_name:
                in_names.append(name)
        elif alloc.kind == "ExternalOutput":
            out_names.append(name)
            out_avals.append(
                jax.core.ShapedArray(
                    tuple(alloc.tensor_shape), mybir.dt.np(alloc.dtype)
                )
            )
    in_names_all = list(in_names) + list(out_names)
    if partition_name is not None:
        in_names_all.append(partition_name)
    n_params = len(in_names)
    n_outs = len(out_avals)

    def _body(*args):
        operands = list(args)
        if partition_name is not None:
            operands.append(partition_id_tensor())
        outs = _bass_exec_p.bind(
            *operands,
            out_avals=tuple(out_avals),
            in_names=tuple(in_names_all),
            out_names=tuple(out_names),
            lowering_input_output_aliases=(),
            sim_require_finite=True,
            sim_require_nnan=True,
            nc=nc,
        )
        return tuple(outs)

    devices = jax.devices()[:NC]
    mesh = Mesh(np.asarray(devices), ("core",))
    shard = NamedSharding(mesh, PartitionSpec("core"))
    sharded = jax.jit(
        shard_map(
            _body, mesh=mesh,
            in_specs=(PartitionSpec("core"),) * (n_params + n_outs),
            out_specs=(PartitionSpec("core"),) * n_outs,
            check_rep=False,
        ),
        donate_argnums=tuple(range(n_params, n_params + n_outs)),
        keep_unused=True,
    )
    zero_shapes = [(NC * a.shape[0], *a.shape[1:]) for a in out_avals]
    zero_dtypes = [a.dtype for a in out_avals]
    mkz = jax.jit(
        lambda: tuple(jnp.zeros(s, d) for s, d in zip(zero_shapes, zero_dtypes)),
        out_shardings=tuple(shard for _ in zero_shapes),
    )

    def runner(in_maps):
        import jax as _jax
        concat_in = [
            _jax.device_put(
                np.concatenate(
                    [np.asarray(in_maps[c][nm]) for c in range(NC)], axis=0
                ),
                shard,
            )
            for nm in in_names
        ]
        outs = sharded(*concat_in, *mkz())
        _jax.block_until_ready(outs)
        per_core = []
        for c in range(NC):
            d = {}
            for i, nm in enumerate(out_names):
                d[nm] = np.asarray(outs[i]).reshape(NC, *out_avals[i].shape)[c]
            per_core.append(d)
        return per_core

    _CACHE["runner"] = runner
    _CACHE["internals"] = dict(
        sharded=sharded, mkz=mkz, in_names=in_names, out_names=out_names,
        out_avals=out_avals, shard=shard,
    )
    return runner


class _Res:
    def __init__(self, results):
        self.results = results
        self.exec_time_ns = None


def run(in_maps, trace=False):
    try:
        return _Res(_get_runner()(in_maps))
    except Exception:
        if "nc" not in _CACHE:
            _CACHE["nc"] = _build_bass()
        return run_bass_kernel_spmd(
            _CACHE["nc"], in_maps, core_ids=list(range(NC)), trace=trace
        )


def kernel(alphas, listRs, listTs, xyzs_in_base_frame, RTs_in_base_frame,
           seq, base_indices):
    in_maps = prepare_in_maps(
        alphas, listRs, listTs, xyzs_in_base_frame, RTs_in_base_frame,
        seq, base_indices,
    )
    res = run(in_maps).results

    fr = np.concatenate(
        [r["frames"][:LCORE].reshape(LCORE, 9, 4, 4) for r in res], axis=0
    )
    xy = np.concatenate(
        [r["xyzo"][:LCORE].reshape(LCORE, 34, 3) for r in res], axis=0
    )
    return fr[None], xy[None]


# revision 25
# speedup vs baseline: 671.7658x; 1.2651x over previous
"""Trainium2 Bass kernel for ComputeAllAtomCoords (nn_ComputeAllAtomCoords).

Full inputs in, full outputs out. Internally: data-parallel over residues L
across 8 NeuronCores (sharding hint). Per core:
  - residues laid out SoA: residue = (partition p, free col f), l = p*F + f
  - torsion rotX normalization on DVE+ACT
  - per-residue RT matrices gathered from the 5-entry table via one-hot
    PE matmul (+PE transpose back to SoA layout)
  - frame chain RTF_k = parent @ RT_k @ rotX_k on DVE with broadcast-AP
    fused tensor_tensor ops
  - atom coords via 5 seq-masked PE matmuls with constant weights
    (xb points and base-frame indices baked into the weights host-side)
"""

import sys

sys.path.insert(0, "/opt/trn_rl_repo")

import numpy as np

import concourse.bass as bass
import concourse.bacc as bacc
import concourse.mybir as mybir
from concourse.bass_utils import run_bass_kernel_spmd
from concourse.tile import TileContext

F32 = mybir.dt.float32
OP = mybir.AluOpType

P = 128          # partitions
F = 49           # free columns per tile
NT = 4           # tiles per core
PF = P * F       # residues per tile (6272)
LCORE_PAD = PF * NT   # 25088
L = 200000
NC = 8
LCORE = L // NC  # 25000
EPS = 1e-6

# chain: frames 1..8 = step(parent_state, torsion k=sidx+2)
PARENT = [0, 1, 2, 3, 3, 5, 6, 6]

_CACHE = {}


def _build_bass():
    nc = bacc.Bacc()

    # DRAM I/O (per-core shapes)
    alph = nc.dram_tensor("alph", [LCORE_PAD, 16], F32, kind="ExternalInput")
    rst = nc.dram_tensor("rst", [LCORE_PAD, 12], F32, kind="ExternalInput")
    seqp = nc.dram_tensor("seqp", [NT, 5, PF], F32, kind="ExternalInput")
    rtt = nc.dram_tensor("rtt", [5, 128], F32, kind="ExternalInput")
    wat = nc.dram_tensor("wat", [108, 510], F32, kind="ExternalInput")
    mskw = nc.dram_tensor("mskw", [5, 540], F32, kind="ExternalInput")
    iden = nc.dram_tensor("iden", [128, 128], F32, kind="ExternalInput")
    iot5 = nc.dram_tensor("iot5", [5, 1], F32, kind="ExternalInput")
    frames = nc.dram_tensor("frames", [LCORE_PAD, 144], F32, kind="ExternalOutput")
    xyzo = nc.dram_tensor("xyzo", [LCORE_PAD, 102], F32, kind="ExternalOutput")

    with TileContext(nc) as tc:
        with (
            tc.tile_pool(name="const", bufs=1) as cpool,
            tc.tile_pool(name="big", bufs=1) as bpool,
            tc.tile_pool(name="ld", bufs=2) as lpool,
            tc.tile_pool(name="small", bufs=2) as spool,
            tc.tile_pool(name="blk", bufs=2) as kpool,
            tc.tile_pool(name="ft", bufs=2) as fpool,
            tc.tile_pool(name="psum", bufs=1, space="PSUM") as ppool,
            tc.tile_pool(name="psum2", bufs=2, space="PSUM") as ppool2,
            tc.tile_pool(name="psum3", bufs=2, space="PSUM") as ppool3,
        )            :
            # --- constants into SBUF ---
            rtt_sb = cpool.tile([5, 128], F32, tag="rtt")
            nc.sync.dma_start(rtt_sb[:], rtt[:, :])
            wat_sb = cpool.tile([108, 510], F32, tag="wat")
            nc.sync.dma_start(wat_sb[:], wat[:, :])
            mskw_sb = cpool.tile([5, 540], F32, tag="mskw")
            nc.sync.dma_start(mskw_sb[:], mskw[:, :])
            iden_sb = cpool.tile([128, 128], F32, tag="iden")
            nc.sync.dma_start(iden_sb[:], iden[:, :])
            iot5_sb = cpool.tile([5, 1], F32, tag="iot5")
            nc.sync.dma_start(iot5_sb[:], iot5[:, :])

            for t in range(NT):
                r0 = t * PF
                # ---- loads ----
                ang = lpool.tile([P, F, 16], F32, tag="ang")
                nc.sync.dma_start(
                    ang[:],
                    alph[r0 : r0 + PF, :].rearrange("(p f) c -> p f c", f=F),
                )
                # states: (P, F, 9, 16) — frame k comps contiguous per residue
                states = bpool.tile([P, F, 9, 16], F32, tag="states")
                rstg = lpool.tile([P, F, 12], F32, tag="rstg")
                nc.sync.dma_start(
                    rstg[:], rst[r0 : r0 + PF, :].rearrange("(p f) c -> p f c", f=F)
                )
                st0r = states[:, :, 0, :].rearrange("p f (i m) -> p f i m", m=4)
                rr = rstg[:, :, 0:9].rearrange("p f (i j) -> p f i j", j=3)
                nc.vector.tensor_copy(st0r[:, :, 0:3, 0:3], rr)
                nc.vector.tensor_copy(st0r[:, :, 0:3, 3], rstg[:, :, 9:12])
                nc.gpsimd.memset(states[:, :, 0, 12:15], 0.0)
                nc.gpsimd.memset(states[:, :, 0, 15:16], 1.0)

                oht = bpool.tile([5, PF], F32, tag="oht")
                nc.sync.dma_start(oht[:], seqp[t])

                # ---- rotX normalization ----
                c_ap = ang[:, :, 0:16:2]
                s_ap = ang[:, :, 1:16:2]
                sq = spool.tile([P, F, 8], F32, tag="sq")
                n2 = spool.tile([P, F, 8], F32, tag="n2")
                rec = spool.tile([P, F, 8], F32, tag="rec")
                nc.vector.tensor_tensor(sq[:], s_ap, s_ap, OP.mult)
                nc.vector.tensor_tensor(n2[:], c_ap, c_ap, OP.mult)
                nc.vector.tensor_tensor(n2[:], n2[:], sq[:], OP.add)
                nc.scalar.sqrt(sq[:], n2[:])
                nc.vector.tensor_scalar_add(sq[:], sq[:], EPS)
                nc.vector.reciprocal(rec[:], sq[:])
                angn = bpool.tile([P, F, 16], F32, tag="angn")
                nc.vector.tensor_tensor(angn[:, :, 0:16:2], c_ap, rec[:], OP.mult)
                nc.vector.tensor_tensor(angn[:, :, 1:16:2], s_ap, rec[:], OP.mult)

                # ---- one-hot over seq (5, PF), in place ----
                nc.vector.tensor_scalar(
                    oht[:], oht[:], iot5_sb[:, 0:1], None, OP.is_equal
                )

                # ---- RTbar gather: (5->128) matmul + transpose to SoA ----
                rtbar = bpool.tile([P, F, 8, 16], F32, tag="rtbar")
                nblk = (F + 3) // 4
                for b in range(nblk):
                    nf = min(4, F - b * 4)
                    nb = nf * 128
                    rb = ppool.tile([128, 512], F32, tag="rb")
                    nc.tensor.matmul(
                        rb[:, :nb],
                        rtt_sb[:],
                        oht[:, b * 512 : b * 512 + nb],
                        start=True,
                        stop=True,
                    )
                    rbs = kpool.tile([128, 512], F32, tag="rbs")
                    nc.scalar.copy(rbs[:, :nb], rb[:, :nb])
                    for fi in range(nf):
                        f_abs = b * 4 + fi
                        tp = ppool.tile([128, 128], F32, tag="tp")
                        nc.tensor.transpose(
                            tp[:], rbs[:, fi * 128 : (fi + 1) * 128], iden_sb[:]
                        )
                        nc.scalar.copy(
                            rtbar[:, f_abs],
                            tp[:].rearrange("p (k c) -> p k c", c=16),
                        )

                # ---- chain ----
                # feat: contiguous rows 0-2 of every frame, for the PE bridge
                feat = fpool.tile([P, F, 108], F32, tag="feat")
                ftr = feat[:, :, 0:12].rearrange("p f (i j) -> p f i j", j=4)
                nc.scalar.copy(ftr[:, :, :, 0:3], rr)
                nc.scalar.copy(ftr[:, :, :, 3], rstg[:, :, 9:12])
                for sidx in range(8):
                    Pst = states[:, :, PARENT[sidx]].rearrange(
                        "p f (i m) -> p f i m", m=4
                    )
                    C4 = states[:, :, sidx + 1].rearrange("p f (i m) -> p f i m", m=4)
                    B4 = rtbar[:, :, sidx, :].rearrange("p f (j m) -> p f j m", m=4)
                    tmp4a = spool.tile([P, F, 4, 4], F32, tag="tmp4a")
                    tmp4b = spool.tile([P, F, 4, 4], F32, tag="tmp4b")
                    for j in range(4):
                        pjb = Pst[:, :, :, j].unsqueeze(3).broadcast_to([P, F, 4, 4])
                        bjb = B4[:, :, j, :].unsqueeze(2).broadcast_to([P, F, 4, 4])
                        if j == 0:
                            nc.vector.tensor_tensor(C4[:], pjb, bjb, OP.mult)
                        else:
                            tmp4 = tmp4a if j % 2 else tmp4b
                            nc.vector.tensor_tensor(tmp4[:], pjb, bjb, OP.mult)
                            nc.vector.tensor_tensor(C4[:], C4[:], tmp4[:], OP.add)
                    # right-multiply by rotX(sidx): cols 1,2 mix
                    cb = angn[:, :, 2 * sidx].unsqueeze(2).broadcast_to([P, F, 4])
                    sb = angn[:, :, 2 * sidx + 1].unsqueeze(2).broadcast_to([P, F, 4])
                    d1 = C4[:, :, :, 1]
                    d2 = C4[:, :, :, 2]
                    ta = spool.tile([P, F, 4], F32, tag="ta")
                    tb2 = spool.tile([P, F, 4], F32, tag="tb2")
                    tcc = spool.tile([P, F, 4], F32, tag="tcc")
                    td = spool.tile([P, F, 4], F32, tag="td")
                    nc.vector.tensor_tensor(ta[:], d1, cb, OP.mult)
                    nc.vector.tensor_tensor(tb2[:], d2, sb, OP.mult)
                    nc.vector.tensor_tensor(tcc[:], d2, cb, OP.mult)
                    nc.vector.tensor_tensor(td[:], d1, sb, OP.mult)
                    nc.vector.tensor_tensor(d1, ta[:], tb2[:], OP.add)
                    nc.vector.tensor_tensor(d2, tcc[:], td[:], OP.subtract)
                    k = sidx + 1
                    nc.scalar.copy(
                        feat[:, :, k * 12 : (k + 1) * 12],
                        states[:, :, k, 0:12],
                    )

                # ---- frames out ----
                nc.sync.dma_start(
                    frames[r0 : r0 + PF, :].rearrange("(p f) c -> p f c", f=F),
                    states[:].rearrange("p f k c -> p f (k c)"),
                )

                # ---- atoms ----
                xyzout = bpool.tile([P, F, 102], F32, tag="xyzout")
                for b in range(nblk):
                    nf = min(4, F - b * 4)
                    nb = nf * 128
                    tb = ppool.tile([108, 512], F32, tag="tb")
                    for fi in range(nf):
                        f_abs = b * 4 + fi
                        nc.tensor.transpose(
                            tb[:, fi * 128 : (fi + 1) * 128],
                            feat[:, f_abs, :],
                            iden_sb[:],
                        )
                    tsb = kpool.tile([108, 512], F32, tag="tsb")
                    nc.scalar.copy(tsb[:, :nb], tb[:, :nb])
                    xy = ppool.tile([102, 512], F32, tag="xy")
                    for s in range(5):
                        ms = ppool2.tile([108, 512], F32, tag="ms")
                        nc.tensor.matmul(
                            ms[:, :nb],
                            mskw_sb[:, s * 108 : (s + 1) * 108],
                            oht[:, b * 512 : b * 512 + nb],
                            start=True,
                            stop=True,
                        )
                        ph = kpool.tile([108, 512], F32, tag="ph")
                        nc.vector.tensor_tensor(
                            ph[:, :nb], tsb[:, :nb], ms[:, :nb], OP.mult
                        )
                        nc.tensor.matmul(
                            xy[:, :nb],
                            wat_sb[:, s * 102 : (s + 1) * 102],
                            ph[:, :nb],
                            start=(s == 0),
                            stop=(s == 4),
                        )
                    xys = kpool.tile([102, 512], F32, tag="xys")
                    nc.scalar.copy(xys[:, :nb], xy[:, :nb])
                    for fi in range(nf):
                        f_abs = b * 4 + fi
                        xt = ppool3.tile([128, 102], F32, tag="xt")
                        nc.tensor.transpose(
                            xt[:],
                            xys[:, fi * 128 : (fi + 1) * 128],
                            iden_sb[0:102, 0:102],
                        )
                        nc.scalar.copy(xyzout[:, f_abs, :], xt[:])

                nc.sync.dma_start(
                    xyzo[r0 : r0 + PF, :].rearrange("(p f) c -> p f c", f=F),
                    xyzout[:],
                )

    nc.finalize()
    return nc


def _host_consts(xyzs_in_base_frame, RTs_in_base_frame, base_indices):
    xb = np.asarray(xyzs_in_base_frame, dtype=np.float32)      # (5,34,4)
    rts = np.asarray(RTs_in_base_frame, dtype=np.float32)      # (5,10,4,4)
    bi = np.asarray(base_indices).astype(np.int64)             # (5,34)

    rtt = np.zeros((5, 128), np.float32)
    for k2 in range(8):
        rtt[:, k2 * 16 : (k2 + 1) * 16] = rts[:, k2 + 2].reshape(5, 16)

    wat = np.zeros((108, 510), np.float32)
    for s in range(5):
        for a in range(34):
            k = int(bi[s, a])
            for i in range(3):
                for j in range(4):
                    wat[k * 12 + i * 4 + j, s * 102 + a * 3 + i] = xb[s, a, j]

    mskw = np.zeros((5, 540), np.float32)
    for s in range(5):
        mskw[s, s * 108 : (s + 1) * 108] = 1.0

    iden = np.eye(128, dtype=np.float32)
    iot5 = np.arange(5, dtype=np.float32).reshape(5, 1)
    return rtt, wat, mskw, iden, iot5


def prepare_in_maps(alphas, listRs, listTs, xyzs_in_base_frame,
                    RTs_in_base_frame, seq, base_indices):
    al = np.asarray(alphas, dtype=np.float32).reshape(L, 10, 2)[:, 2:10, :]
    al = np.ascontiguousarray(al).reshape(L, 16)
    rs = np.asarray(listRs, dtype=np.float32).reshape(L, 9)
    ts = np.asarray(listTs, dtype=np.float32).reshape(L, 3)
    rst = np.concatenate([rs, ts], axis=1)                     # (L,12)
    sq = np.asarray(seq).astype(np.float32)                    # (L,)

    rtt, wat, mskw, iden, iot5 = _host_consts(
        xyzs_in_base_frame, RTs_in_base_frame, base_indices
    )

    in_maps = []
    for c in range(NC):
        lo = c * LCORE
        a_pad = np.zeros((LCORE_PAD, 16), np.float32)
        a_pad[:LCORE] = al[lo : lo + LCORE]
        r_pad = np.zeros((LCORE_PAD, 12), np.float32)
        r_pad[:LCORE] = rst[lo : lo + LCORE]
        s_pad = np.zeros((LCORE_PAD,), np.float32)
        s_pad[:LCORE] = sq[lo : lo + LCORE]
        # seqp[t, :, f*128 + p] = seq[t*PF + p*F + f], replicated on 5 rows
        sq1 = s_pad.reshape(NT, P, F).transpose(0, 2, 1).reshape(NT, 1, PF)
        seqp = np.broadcast_to(sq1, (NT, 5, PF)).copy()
        in_maps.append(
            dict(alph=a_pad, rst=r_pad, seqp=seqp, rtt=rtt, wat=wat,
                 mskw=mskw, iden=iden, iot5=iot5)
        )
    return in_maps


def _get_runner():
    """Cached jitted PJRT runner (same execution path run_bass_kernel_spmd
    takes under axon, but built once so repeated kernel() calls don't
    re-trace/re-compile)."""
    if "runner" in _CACHE:
        return _CACHE["runner"]
    import jax
    import jax.numpy as jnp
    from jax.sharding import Mesh, PartitionSpec, NamedSharding
    from jax.experimental.shard_map import shard_map
    from concourse.bass2jax import (
        _bass_exec_p, install_neuronx_cc_hook, partition_id_tensor,
    )

    if "nc" not in _CACHE:
        _CACHE["nc"] = _build_bass()
    nc = _CACHE["nc"]
    install_neuronx_cc_hook()
    partition_name = nc.partition_id_tensor.name if nc.partition_id_tensor else None
    in_names, out_names, out_avals = [], [], []
    for alloc in nc.m.functions[0].allocations:
        if not isinstance(alloc, mybir.MemoryLocationSet):
            continue
        name = alloc.memorylocations[0].name
        if alloc.kind == "ExternalInput":
            if name != partition_name:
                in_names.append(name)
        elif alloc.kind == "ExternalOutput":
            out_names.append(name)
            out_avals.append(
                jax.core.ShapedArray(
                    tuple(alloc.tensor_shape), mybir.dt.np(alloc.dtype)
                )
            )
    in_names_all = list(in_names) + list(out_names)
    if partition_name is not None:
        in_names_all.append(partition_name)
    n_params = len(in_names)
    n_outs = len(out_avals)

    def _body(*args):
        operands = list(args)
        if partition_name is not None:
            operands.append(partition_id_tensor())
        outs = _bass_exec_p.bind(
            *operands,
            out_avals=tuple(out_avals),
            in_names=tuple(in_names_all),
            out_names=tuple(out_names),
            lowering_input_output_aliases=(),
            sim_require_finite=True,
            sim_require_nnan=True,
            nc=nc,
        )
        return tuple(outs)

    devices = jax.devices()[:NC]
    mesh = Mesh(np.asarray(devices), ("core",))
    shard = NamedSharding(mesh, PartitionSpec("core"))
    sharded = jax.jit(
        shard_map(
            _body, mesh=mesh,
            in_specs=(PartitionSpec("core"),) * (n_params + n_outs),
            out_specs=(PartitionSpec("core"),) * n_outs,
            check_rep=False,
        ),
        donate_argnums=tuple(range(n_params, n_params + n_outs)),
        keep_unused=True,
    )
    zero_shapes = [(NC * a.shape[0], *a.shape[1:]) for a in out_avals]
    zero_dtypes = [a.dtype for a in out_avals]
    mkz = jax.jit(
        lambda: tuple(jnp.zeros(s, d) for s, d in zip(zero_shapes, zero_dtypes)),
        out_shardings=tuple(shard for _ in zero_shapes),
    )

    def runner(in_maps):
        import jax as _jax
        concat_in = [
            _jax.device_put(
                np.concatenate(
                    [np.asarray(in_maps[c][nm]) for c in range(NC)], axis=0
                ),
                shard,
            )
            for nm in in_names
        ]
        outs = sharded(*concat_in, *mkz())
        _jax.block_until_ready(outs)
        per_core = []
        for c in range(NC):
            d = {}
            for i, nm in enumerate(out_names):
                d[nm] = np.asarray(outs[i]).reshape(NC, *out_avals[i].shape)[c]
            per_core.append(d)
        return per_core

    _CACHE["runner"] = runner
    _CACHE["internals"] = dict(
        sharded=sharded, mkz=mkz, in_names=in_names, out_names=out_names,
        out_avals=out_avals, shard=shard,
    )
    return runner


class _Res:
    def __init__(self, results):
        self.results = results
        self.exec_time_ns = None


def run(in_maps, trace=False):
    try:
        return _Res(_get_runner()(in_maps))
    except Exception:
        if "nc" not in _CACHE:
            _CACHE["nc"] = _build_bass()
        return run_bass_kernel_spmd(
            _CACHE["nc"], in_maps, core_ids=list(range(NC)), trace=trace
        )


def kernel(alphas, listRs, listTs, xyzs_in_base_frame, RTs_in_base_frame,
           seq, base_indices):
    in_maps = prepare_in_maps(
        alphas, listRs, listTs, xyzs_in_base_frame, RTs_in_base_frame,
        seq, base_indices,
    )
    res = run(in_maps).results

    fr = np.concatenate(
        [r["frames"][:LCORE].reshape(LCORE, 9, 4, 4) for r in res], axis=0
    )
    xy = np.concatenate(
        [r["xyzo"][:LCORE].reshape(LCORE, 34, 3) for r in res], axis=0
    )
    return fr[None], xy[None]
